# revision 4
# baseline (speedup 1.0000x reference)
"""DifferentialCausalAttention on 8 Trainium2 NeuronCores.

Sharding: 8 cores = 2 batches x 4 head-groups (tensor-parallel over heads).
Core c handles batch b = c // 4 and head-group g = c % 4:
  - query heads 8g..8g+7 (4 pairs), kv heads 4g..4g+3, lambda cols 4g..4g+3
  - W_O rows 512g..512g+511 -> partial output, host-summed over the 4 groups.

Layouts on device: Q^T/K^T as [dh, L] (dh on partitions), V as [L, d],
attention computed transposed (S^T = [k, q]) so no P-transposes are needed.
Q^T/K^T and V stay SBUF-resident between the projection and attention
phases (no DRAM scratch round-trip). Weights are host-packed so every
weight DMA moves 4KB-contiguous per-partition lines. Softmax denominators
are accumulated on the vector engine (element-wise over k-chunks) with a
single ones-matmul per superblock, instead of a PE rowsum pass per chunk.
"""
import os
from contextlib import ExitStack

import ml_dtypes
import numpy as np

import concourse.bass as bass
import concourse.mybir as mybir
import concourse.tile as tile
from concourse import bacc
from concourse.bass_utils import run_bass_kernel_spmd

F32 = mybir.dt.float32
F32R = mybir.dt.float32r
BF16 = mybir.dt.bfloat16

ATT_BF16 = os.environ.get("ATT_BF16", "1") == "1"   # phase-2 S/ctx operands
P1_BF16 = os.environ.get("P1_BF16", "1") == "1"     # phase-1 projection operands
P3_BF16 = os.environ.get("P3_BF16", "1") == "1"     # phase-3 Wo/diffT
OUT_F32 = os.environ.get("OUT_F32", "0") == "1"     # outT dtype (bf16 default)
DT_ATT = BF16 if ATT_BF16 else F32R
DT_P1 = BF16 if P1_BF16 else F32R
DT_P3 = BF16 if P3_BF16 else F32R
DT_OUT = F32 if OUT_F32 else BF16
NP_P1 = ml_dtypes.bfloat16 if P1_BF16 else np.float32
NP_ATT = ml_dtypes.bfloat16 if ATT_BF16 else np.float32
NP_P3 = ml_dtypes.bfloat16 if P3_BF16 else np.float32

B, L, D, NH = 2, 2048, 2048, 16
DH = D // NH            # 128
G = 4                   # head groups (cores per batch)
NKV = NH // G           # kv heads per core = 4
NQ = 2 * NKV            # query heads per core = 8
CQK = NQ * DH + NKV * DH  # 1536 projection cols (Q then K)
CT = CQK // 128         # 12 column tiles (0-7 Q heads, 8-11 K heads)
DC = D // 128           # 16 contraction chunks
LCH = L // 512          # 4 L-chunks
LT = L // 128           # 16 L-tiles / q-tiles
SCALE = 1.0 / float(np.sqrt(DH))
ROPE_BASE = 10000.0


def build_kernel() -> bacc.Bacc:
    nc = bacc.Bacc("TRN2", target_bir_lowering=False, debug=False)

    xT = nc.dram_tensor("xT", [D, L], DT_P1, kind="ExternalInput")
    Wqkp = nc.dram_tensor("Wqkp", [128, CT, DC * 128], DT_P1, kind="ExternalInput")
    Wvp = nc.dram_tensor("Wvp", [128, DC * NKV * DH], DT_P1, kind="ExternalInput")
    Wlp = nc.dram_tensor("Wlp", [128, DC * 128], DT_P1, kind="ExternalInput")
    blv = nc.dram_tensor("blv", [128, 1], F32, kind="ExternalInput")
    Wop = nc.dram_tensor("Wop", [128, NKV, D], DT_P3, kind="ExternalInput")
    cosT = nc.dram_tensor("cosT", [DH, L], F32, kind="ExternalInput")
    sinTs = nc.dram_tensor("sinTs", [DH, L], F32, kind="ExternalInput")
    maskT = nc.dram_tensor("maskT", [128, 256], DT_ATT, kind="ExternalInput")
    onesin = nc.dram_tensor("onesin", [128, 128], F32R, kind="ExternalInput")
    outT = nc.dram_tensor("outT", [D, L], DT_OUT, kind="ExternalOutput")
    dbg = None
    if os.environ.get("KDBG") == "1":
        dbg = nc.dram_tensor("dbg", [20, L], F32, kind="ExternalOutput")

    with ExitStack() as ctx:
        tc = ctx.enter_context(tile.TileContext(nc))

        persist = ctx.enter_context(tc.tile_pool(name="persist", bufs=1))

        # ---- persistent tiles ----
        mask_sb = persist.tile([128, 256], DT_ATT)
        ones_sb = persist.tile([128, 128], F32R)
        bl_sb = persist.tile([128, 1], F32)
        wo_sb = persist.tile([128, NKV, D], DT_P3)
        lam_sb = persist.tile([NKV, L], F32)           # sigmoid(x@Wl+bl), row per kv head
        diffT = persist.tile([128, NKV, L], DT_P3)     # (ctx0 - lam*ctx1)^T per head
        qkT_sb = persist.tile([128, CT, L], DT_ATT)    # Q^T/K^T after RoPE (resident)
        v_res = persist.tile([128, LT, NKV * DH], DT_ATT)  # V as [k%128, k//128, d]

        # ================= Phase 1: projections + RoPE =================
        with tc.tile_pool(name="ph1", bufs=1) as ph1, \
                tc.tile_pool(name="ps1", bufs=1, space="PSUM") as ps1:
            xTr = xT.rearrange("(dc p) l -> p dc l", p=128)
            wv_sb = ph1.tile([128, DC * NKV * DH], DT_P1)
            wl_sb = ph1.tile([128, DC * 128], DT_P1)

            wt_pre = None
            for lch in range(LCH):
                ls = slice(lch * 512, (lch + 1) * 512)
                if lch == 0:
                    # first weight tile ahead of the x burst so the PE can
                    # start as early as possible
                    wt_pre = ph1.tile([128, DC * 128], DT_P1, name="wt", tag="wt", bufs=3)
                    nc.sync.dma_start(wt_pre[:], Wqkp[:, 0, :])
                xs = ph1.tile([128, DC, 512], DT_P1, name="xs", tag="xs", bufs=2)
                for dc in range(DC):
                    nc.sync.dma_start(xs[:, dc, :], xTr[:, dc, ls])
                cos_sb = ph1.tile([128, 512], F32, name="cos_sb", tag="cos", bufs=2)
                sin_sb = ph1.tile([128, 512], F32, name="sin_sb", tag="sin", bufs=2)
                if os.environ.get("NO_TAB") != "1":
                    nc.sync.dma_start(cos_sb[:], cosT[:, ls])
                    nc.sync.dma_start(sin_sb[:], sinTs[:, ls])

                # --- Q^T / K^T column tiles + RoPE ---
                for ct in range(CT):
                    if lch == 0 and ct == 0:
                        wt = wt_pre
                    else:
                        wt = ph1.tile([128, DC * 128], DT_P1, name="wt", tag="wt", bufs=3)
                        nc.sync.dma_start(wt[:], Wqkp[:, ct, :])
                    qk_ps = ps1.tile([128, 512], F32, name="qk_ps", tag="mm512", bufs=4)
                    for dc in range(DC):
                        nc.tensor.matmul(
                            qk_ps[:], wt[:, dc * 128:(dc + 1) * 128], xs[:, dc, :],
                            start=(dc == 0), stop=(dc == DC - 1),
                        )
                    # RoPE: qr = qk*cos + rot(qk)*sin_signed, written resident
                    if os.environ.get("NO_ROPE") == "1":
                        nc.scalar.copy(qkT_sb[:, ct, ls], qk_ps[:])
                    else:
                        rot = ph1.tile([128, 512], F32, name="rot", tag="rot", bufs=2)
                        nc.scalar.copy(rot[0:64, :], qk_ps[64:128, :])
                        nc.scalar.copy(rot[64:128, :], qk_ps[0:64, :])
                        t1 = ph1.tile([128, 512], F32, name="t1", tag="t1", bufs=2)
                        nc.vector.tensor_mul(t1[:], qk_ps[:], cos_sb[:])
                        t2 = ph1.tile([128, 512], F32, name="t2", tag="t2", bufs=2)
                        nc.vector.tensor_mul(t2[:], rot[:], sin_sb[:])
                        nc.vector.tensor_add(qkT_sb[:, ct, ls], t1[:], t2[:])
                    if lch == 0 and ct == 0:
                        # aux/persistent loads trickle on the gpsimd DMA ring,
                        # off the hot sync ring carrying wt/xs
                        nc.gpsimd.dma_start(wv_sb[:], Wvp[:, :])
                        nc.gpsimd.dma_start(wl_sb[:], Wlp[:, :])
                        nc.gpsimd.dma_start(bl_sb[:], blv[:, :])
                        nc.gpsimd.dma_start(mask_sb[:], maskT[:, :])
                        nc.gpsimd.dma_start(ones_sb[:], onesin[:, :])
                        nc.gpsimd.dma_start(wo_sb[:], Wop[:, :, :])

                # --- V tiles ---
                for lt in ([] if os.environ.get("NO_V") == "1" else range(4)):
                    v_ps = ps1.tile([128, 512], F32, name="v_ps", tag="mm512", bufs=4)
                    for dc in range(DC):
                        nc.tensor.matmul(
                            v_ps[:], xs[:, dc, lt * 128:(lt + 1) * 128],
                            wv_sb[:, dc * 512:(dc + 1) * 512],
                            start=(dc == 0), stop=(dc == DC - 1),
                        )
                    nc.scalar.copy(v_res[:, lch * 4 + lt, :], v_ps[:])

                # --- lambda ---
                if os.environ.get("NO_LAM") == "1":
                    continue
                lam_ps = ps1.tile([128, 512], F32, name="lam_ps", tag="mm512", bufs=4)
                for dc in range(DC):
                    nc.tensor.matmul(
                        lam_ps[:], wl_sb[:, dc * 128:(dc + 1) * 128], xs[:, dc, :],
                        start=(dc == 0), stop=(dc == DC - 1),
                    )
                nc.scalar.activation(
                    lam_sb[:, ls], lam_ps[0:NKV, :],
                    mybir.ActivationFunctionType.Sigmoid, bias=bl_sb[0:NKV, 0:1],
                )

        if dbg is not None and os.environ.get("NO_LAM") != "1":
            nc.sync.dma_start(dbg[0:NKV, :], lam_sb[:, :])
        trunc = os.environ.get("KTRUNC") == "1"
        # ================= Phase 2: causal attention per head pair =================
        # Two q-tiles (a "superblock": A=2sb, B=2sb+1) are processed at once so
        # every moving operand is 512 wide: columns ordered (qtile, head, l) =
        # [A.h0 | A.h1 | B.h0 | B.h1]. Per k-chunk: one S matmul [128,512], one
        # exp, one ctx matmul. Chunk kc==A is full width but masks its [0:256]
        # half; chunk kc==B covers only [256:512]. ctx for chunk kc is emitted
        # after the S matmul of chunk kc+2 (PE never waits on ACT). The softmax
        # denominator is accumulated on DVE (esum += exp chunk); one ones-matmul
        # per superblock (deferred into the next superblock) reduces it over
        # partitions, and the normalization tail is deferred one more group.
        with tc.tile_pool(name="ph2", bufs=1) as ph2, \
                tc.tile_pool(name="ps2", bufs=1, space="PSUM") as ps2:
            pend_rs = []
            pend_norm = []

            def emit_rs(st):
                esum_, ctx_ps, qtA_, p_, lam0_ = st
                rs_ps = ps2.tile([1, 512], F32, name="rs_ps", tag="small", bufs=1)
                nc.tensor.matmul(
                    rs_ps[0:1, :], ones_sb[:, 0:1], esum_[:],
                    start=True, stop=True, skip_group_check=True,
                )
                rs_sb = ph2.tile([1, 512], F32, name="rs_sb", tag="rs_sb", bufs=3)
                nc.vector.tensor_copy(rs_sb[:], rs_ps[:])
                pend_norm.append((ctx_ps, rs_sb, qtA_, p_, lam0_))

            def emit_norm(st):
                ctx_ps, rs_sb, qtA_, p_, lam0_ = st
                recip = ph2.tile([1, 2, 256], F32, name="recip", tag="recip", bufs=2)
                nc.vector.reciprocal_approx_fast(
                    recip.rearrange("p t l -> p (t l)"), rs_sb[:]
                )
                cs = ph2.tile([1, 2, 256], F32R, name="cs", tag="cs", bufs=2)
                nc.vector.tensor_copy(cs[:, :, 0:128], recip[:, :, 0:128])
                nc.vector.tensor_mul(
                    cs[:, :, 128:256], recip[:, :, 128:256],
                    lam0_[:, qtA_ * 128:(qtA_ + 2) * 128].rearrange(
                        "p (t l) -> p t l", t=2
                    ),
                )
                b_ps = ps2.tile([128, 512], F32, name="b_ps", tag="bps", bufs=1)
                nc.tensor.matmul(
                    b_ps[:], ones_sb[0:1, :], cs.rearrange("p t l -> p (t l)"),
                    start=True, stop=True,
                )
                b_sb = ph2.tile([128, 2, 256], F32, name="b_sb", tag="bsb", bufs=2)
                nc.vector.tensor_copy(b_sb.rearrange("p t l -> p (t l)"), b_ps[:])
                ctx3 = ctx_ps.rearrange("p (t l) -> p t l", t=2)
                t0 = ph2.tile([128, 2, 128], F32, name="t0", tag="t0", bufs=2)
                nc.vector.tensor_mul(t0[:], ctx3[:, :, 0:128], b_sb[:, :, 0:128])
                t1b = ph2.tile([128, 2, 128], F32, name="t1b", tag="t1b", bufs=2)
                nc.vector.tensor_mul(t1b[:], ctx3[:, :, 128:256], b_sb[:, :, 128:256])
                nc.vector.tensor_sub(
                    diffT[:, p_, qtA_ * 128:(qtA_ + 2) * 128],
                    t0.rearrange("p t l -> p (t l)"),
                    t1b.rearrange("p t l -> p (t l)"),
                )

            for p in ([] if trunc else range(NKV)):
                lam0 = ph2.tile([1, L], F32, name="lam0", tag="lam0", bufs=2)
                nc.gpsimd.dma_start(lam0[:], lam_sb[p:p + 1, :])

                def emit_block(st, p=p):
                    ctx_ps, e_sb, j, kc, qtB, off, wid = st
                    nc.tensor.matmul(
                        ctx_ps[:, off:off + wid],
                        v_res[:, kc, p * 128:(p + 1) * 128],
                        e_sb[:, j, off:off + wid],
                        start=(kc == 0), stop=(kc == qtB), skip_group_check=True,
                    )

                for sb in range(LT // 2):
                    qtA, qtB = 2 * sb, 2 * sb + 1
                    rhs_full = qkT_sb[
                        :, 2 * p:2 * p + 2, qtA * 128:(qtA + 2) * 128
                    ].rearrange("pp h (t l) -> pp t h l", t=2)
                    rhs_half = qkT_sb[:, 2 * p:2 * p + 2, qtB * 128:(qtB + 1) * 128]
                    ctx_ps = ps2.tile([128, 512], F32, name="ctx_ps", tag="ctx", bufs=2)
                    esum = ph2.tile([128, 512], F32R, name="esum", tag="esum", bufs=2)
                    pend = []
                    # chunk groups of 2 sharing one 2-bank S tile; exp per group
                    groups = []
                    kcs = list(range(qtB + 1))
                    for gi in range(0, len(kcs), 2):
                        groups.append(kcs[gi:gi + 2])
                    for gk, grp in enumerate(groups):
                        s_ps = ps2.tile([128, 2, 512], F32, name="s_ps", tag="s2", bufs=2)
                        segs = []
                        for j, kc in enumerate(grp):
                            off, wid = (256, 256) if kc == qtB else (0, 512)
                            rhs = rhs_half if kc == qtB else rhs_full
                            nc.tensor.matmul(
                                s_ps[:, j, off:off + wid],
                                qkT_sb[:, NQ + p, kc * 128:(kc + 1) * 128],
                                rhs,
                                start=True, stop=True, skip_group_check=True,
                            )
                            segs.append((j, kc, off, wid))
                        # deferred tails from the previous superblock
                        if gk == 0:
                            if pend_rs:
                                emit_rs(pend_rs.pop(0))
                            if len(groups) == 1 and pend_norm:
                                emit_norm(pend_norm.pop(0))
                        elif gk == 1 and pend_norm:
                            emit_norm(pend_norm.pop(0))
                        while len(pend) >= 2:
                            emit_block(pend.pop(0))
                        e_sb = ph2.tile([128, 2, 512], DT_ATT, name="e_sb", tag="e", bufs=3)
                        if len(segs) == 2 and segs[0][3] == 512 and segs[1][3] == 512:
                            nc.scalar.activation(
                                e_sb.rearrange("p a b -> p (a b)"),
                                s_ps.rearrange("p a b -> p (a b)"),
                                mybir.ActivationFunctionType.Exp, scale=SCALE,
                            )
                        else:
                            for j, kc, off, wid in segs:
                                nc.scalar.activation(
                                    e_sb[:, j, off:off + wid], s_ps[:, j, off:off + wid],
                                    mybir.ActivationFunctionType.Exp, scale=SCALE,
                                )
                        for j, kc, off, wid in segs:
                            if kc == qtA:
                                nc.vector.tensor_mul(
                                    e_sb[:, j, 0:256], e_sb[:, j, 0:256], mask_sb[:]
                                )
                            elif kc == qtB:
                                nc.vector.tensor_mul(
                                    e_sb[:, j, 256:512], e_sb[:, j, 256:512], mask_sb[:]
                                )
                            # softmax denominator accumulation on DVE
                            if kc == 0:
                                nc.vector.tensor_copy(esum[:], e_sb[:, j, :])
                            elif kc == qtB:
                                nc.vector.tensor_add(
                                    esum[:, 256:512], esum[:, 256:512],
                                    e_sb[:, j, 256:512],
                                )
                            else:
                                nc.vector.tensor_add(esum[:], esum[:], e_sb[:, j, :])
                            pend.append((ctx_ps, e_sb, j, kc, qtB, off, wid))
                    for st in pend:
                        emit_block(st)
                    pend_rs.append((esum, ctx_ps, qtA, p, lam0))
            while pend_rs:
                emit_rs(pend_rs.pop(0))
            while pend_norm:
                emit_norm(pend_norm.pop(0))

        # ================= Phase 3: output projection =================
        with tc.tile_pool(name="ph3", bufs=1) as ph3, \
                tc.tile_pool(name="ps3", bufs=1, space="PSUM") as ps3:
            for ot in ([] if trunc else range(LT)):
                for qch in range(LCH):
                    o_ps = ps3.tile([128, 512], F32, name="o_ps", tag="mm512", bufs=4)
                    for p in range(NKV):
                        nc.tensor.matmul(
                            o_ps[:],
                            wo_sb[:, p, ot * 128:(ot + 1) * 128],
                            diffT[:, p, qch * 512:(qch + 1) * 512],
                            start=(p == 0), stop=(p == NKV - 1),
                        )
                    o_sb = ph3.tile([128, 512], DT_OUT, name="o_sb", tag="osb", bufs=4)
                    nc.scalar.copy(o_sb[:], o_ps[:])
                    nc.sync.dma_start(
                        outT[ot * 128:(ot + 1) * 128, qch * 512:(qch + 1) * 512], o_sb[:]
                    )

    nc.finalize()
    return nc


def _host_tables():
    half = DH // 2
    inv_freq = 1.0 / (ROPE_BASE ** (np.arange(0, half, dtype=np.float64) * 2.0 / DH))
    freqs = np.arange(L, dtype=np.float64)[:, None] * inv_freq[None, :]  # [L, half]
    emb = np.concatenate([freqs, freqs], axis=-1)  # [L, DH]
    cosT = np.ascontiguousarray(np.cos(emb).T.astype(np.float32))  # [DH, L]
    sinT = np.sin(emb).T.astype(np.float32)
    sinTs = np.concatenate([-sinT[:half], sinT[half:]], axis=0)
    sinTs = np.ascontiguousarray(sinTs.astype(np.float32))
    tri = np.triu(np.ones((128, 128), dtype=np.float32))  # keep k' <= q'
    maskT = np.ascontiguousarray(np.concatenate([tri, tri], axis=1))
    ones = np.ones((128, 128), dtype=np.float32)
    return cosT, sinTs, maskT, ones


def _pack_rows(w, inner):
    """[D, C] -> [128, (DC, C)] with row dc*128+p on partition p, contiguous
    per-partition lines (C = inner)."""
    d, c = w.shape
    assert c == inner and d == D
    return np.ascontiguousarray(
        w.reshape(DC, 128, inner).transpose(1, 0, 2).reshape(128, DC * inner)
    )


_NC_CACHE = []


def kernel(x, Wq, Wk, Wv, Wl, bl, Wo):
    x = np.asarray(x, dtype=np.float32)
    Wq = np.asarray(Wq, dtype=np.float32)
    Wk = np.asarray(Wk, dtype=np.float32)
    Wv = np.asarray(Wv, dtype=np.float32)
    Wl = np.asarray(Wl, dtype=np.float32)
    bl = np.asarray(bl, dtype=np.float32)
    Wo = np.asarray(Wo, dtype=np.float32)

    cosT, sinTs, maskT, ones = _host_tables()
    Wq3 = Wq.reshape(D, 2 * NH, DH)
    Wk3 = Wk.reshape(D, NH, DH)

    in_maps = []
    for c in range(8):
        b, g = divmod(c, G)
        wq_s = Wq3[:, 8 * g:8 * g + NQ, :].reshape(D, NQ * DH)
        wk_s = Wk3[:, G * g:G * g + NKV, :].reshape(D, NKV * DH)
        wqk = np.concatenate([wq_s, wk_s], axis=1).astype(NP_P1)  # [D, CQK]
        # pack: Wqkp[p, ct, dc*128+cc] = wqk[dc*128+p, ct*128+cc]
        wqkp = np.ascontiguousarray(
            wqk.reshape(DC, 128, CT, 128).transpose(1, 2, 0, 3).reshape(128, CT, DC * 128)
        )
        wv_s = Wv[:, DH * G * g:DH * G * g + NKV * DH].astype(NP_P1)
        wl_s = np.pad(Wl[:, G * g:G * g + NKV], ((0, 0), (0, 128 - NKV))).astype(NP_P1)
        wo_s = Wo[512 * g:512 * (g + 1), :].astype(NP_P3)  # [NKV*128, D]
        wop = np.ascontiguousarray(wo_s.reshape(NKV, 128, D).transpose(1, 0, 2))
        in_maps.append({
            "xT": np.ascontiguousarray(x[b].T).astype(NP_P1),
            "Wqkp": wqkp,
            "Wvp": _pack_rows(wv_s, NKV * DH),
            "Wlp": _pack_rows(wl_s, 128),
            "blv": np.ascontiguousarray(np.pad(bl[G * g:G * g + NKV], (0, 128 - NKV)).reshape(128, 1)),
            "Wop": wop,
            "cosT": cosT,
            "sinTs": sinTs,
            "maskT": maskT.astype(NP_ATT),
            "onesin": ones,
        })

    if not _NC_CACHE:
        _NC_CACHE.append(build_kernel())
    nc = _NC_CACHE[0]
    res = run_bass_kernel_spmd(nc, in_maps, core_ids=list(range(8)))

    out = np.empty((B, L, D), dtype=np.float32)
    for b in range(B):
        acc = res.results[4 * b]["outT"].astype(np.float32)
        for g in range(1, G):
            acc += res.results[4 * b + g]["outT"].astype(np.float32)
        out[b] = acc.T
    return out


# revision 7
# speedup vs baseline: 1.0611x; 1.0611x over previous
"""DifferentialCausalAttention on 8 Trainium2 NeuronCores.

Sharding: 8 cores = 2 batches x 4 head-groups (tensor-parallel over heads).
Core c handles batch b = c // 4 and head-group g = c % 4:
  - query heads 8g..8g+7 (4 pairs), kv heads 4g..4g+3, lambda cols 4g..4g+3
  - W_O rows 512g..512g+511 -> partial output, host-summed over the 4 groups.

Layouts on device: Q^T/K^T as [dh, L] (dh on partitions), V as [L, d],
attention computed transposed (S^T = [k, q]) so no P-transposes are needed.
Q^T/K^T and V stay SBUF-resident between the projection and attention
phases (no DRAM scratch round-trip). Weights are host-packed so every
weight DMA moves 4KB-contiguous per-partition lines. Softmax denominators
are accumulated on the vector engine (element-wise over k-chunks) with a
single ones-matmul per superblock, instead of a PE rowsum pass per chunk.
"""
import os
from contextlib import ExitStack

import ml_dtypes
import numpy as np

import concourse.bass as bass
import concourse.mybir as mybir
import concourse.tile as tile
from concourse import bacc
from concourse.bass_utils import run_bass_kernel_spmd

F32 = mybir.dt.float32
F32R = mybir.dt.float32r
BF16 = mybir.dt.bfloat16

ATT_BF16 = os.environ.get("ATT_BF16", "1") == "1"   # phase-2 S/ctx operands
P1_BF16 = os.environ.get("P1_BF16", "1") == "1"     # phase-1 projection operands
P3_BF16 = os.environ.get("P3_BF16", "1") == "1"     # phase-3 Wo/diffT
OUT_F32 = os.environ.get("OUT_F32", "0") == "1"     # outT dtype (bf16 default)
DT_ATT = BF16 if ATT_BF16 else F32R
DT_P1 = BF16 if P1_BF16 else F32R
DT_P3 = BF16 if P3_BF16 else F32R
DT_OUT = F32 if OUT_F32 else BF16
NP_P1 = ml_dtypes.bfloat16 if P1_BF16 else np.float32
NP_ATT = ml_dtypes.bfloat16 if ATT_BF16 else np.float32
NP_P3 = ml_dtypes.bfloat16 if P3_BF16 else np.float32

B, L, D, NH = 2, 2048, 2048, 16
DH = D // NH            # 128
G = 4                   # head groups (cores per batch)
NKV = NH // G           # kv heads per core = 4
NQ = 2 * NKV            # query heads per core = 8
CQK = NQ * DH + NKV * DH  # 1536 projection cols (Q then K)
CT = CQK // 128         # 12 column tiles (0-7 Q heads, 8-11 K heads)
DC = D // 128           # 16 contraction chunks
LCH = L // 512          # 4 L-chunks
LT = L // 128           # 16 L-tiles / q-tiles
SCALE = 1.0 / float(np.sqrt(DH))
ROPE_BASE = 10000.0


def build_kernel() -> bacc.Bacc:
    nc = bacc.Bacc("TRN2", target_bir_lowering=False, debug=False)

    xT = nc.dram_tensor("xT", [D, L], DT_P1, kind="ExternalInput")
    Wqkp = nc.dram_tensor("Wqkp", [128, CT, DC * 128], DT_P1, kind="ExternalInput")
    Wvp = nc.dram_tensor("Wvp", [128, DC * NKV * DH], DT_P1, kind="ExternalInput")
    Wlp = nc.dram_tensor("Wlp", [128, DC * 128], DT_P1, kind="ExternalInput")
    blv = nc.dram_tensor("blv", [128, 1], F32, kind="ExternalInput")
    Wop = nc.dram_tensor("Wop", [128, NKV, D], DT_P3, kind="ExternalInput")
    cosT = nc.dram_tensor("cosT", [DH, L], F32, kind="ExternalInput")
    sinTs = nc.dram_tensor("sinTs", [DH, L], F32, kind="ExternalInput")
    maskT = nc.dram_tensor("maskT", [128, 256], DT_ATT, kind="ExternalInput")
    onesin = nc.dram_tensor("onesin", [128, 128], F32R, kind="ExternalInput")
    outT = nc.dram_tensor("outT", [D, L], DT_OUT, kind="ExternalOutput")
    dbg = None
    if os.environ.get("KDBG") == "1":
        dbg = nc.dram_tensor("dbg", [20, L], F32, kind="ExternalOutput")

    with ExitStack() as ctx:
        tc = ctx.enter_context(tile.TileContext(nc))

        persist = ctx.enter_context(tc.tile_pool(name="persist", bufs=1))

        # ---- persistent tiles ----
        mask_sb = persist.tile([128, 256], DT_ATT)
        ones_sb = persist.tile([128, 128], F32R)
        ones_att = persist.tile([128, 1], DT_ATT)
        bl_sb = persist.tile([128, 1], F32)
        wo_sb = persist.tile([128, NKV, D], DT_P3)
        lam_sb = persist.tile([NKV, L], F32)           # sigmoid(x@Wl+bl), row per kv head
        diffT = persist.tile([128, NKV, L], DT_P3)     # (ctx0 - lam*ctx1)^T per head
        qkT_sb = persist.tile([128, CT, L], DT_ATT)    # Q^T/K^T after RoPE (resident)
        v_res = persist.tile([128, LT, NKV * DH], DT_ATT)  # V as [k%128, k//128, d]

        # ================= Phase 1: projections + RoPE =================
        with tc.tile_pool(name="ph1", bufs=1) as ph1, \
                tc.tile_pool(name="ps1", bufs=1, space="PSUM") as ps1:
            xTr = xT.rearrange("(dc p) l -> p dc l", p=128)
            wv_sb = ph1.tile([128, DC * NKV * DH], DT_P1)
            wl_sb = ph1.tile([128, DC * 128], DT_P1)

            wt_pre = None
            for lch in range(LCH):
                ls = slice(lch * 512, (lch + 1) * 512)
                if lch == 0:
                    # first weight tile ahead of the x burst so the PE can
                    # start as early as possible
                    wt_pre = ph1.tile([128, DC * 128], DT_P1, name="wt", tag="wt", bufs=3)
                    nc.sync.dma_start(wt_pre[:], Wqkp[:, 0, :])
                xs = ph1.tile([128, DC, 512], DT_P1, name="xs", tag="xs", bufs=2)
                for dc in range(DC):
                    nc.sync.dma_start(xs[:, dc, :], xTr[:, dc, ls])
                cos_sb = ph1.tile([128, 512], F32, name="cos_sb", tag="cos", bufs=2)
                sin_sb = ph1.tile([128, 512], F32, name="sin_sb", tag="sin", bufs=2)
                if os.environ.get("NO_TAB") != "1":
                    nc.sync.dma_start(cos_sb[:], cosT[:, ls])
                    nc.sync.dma_start(sin_sb[:], sinTs[:, ls])

                # --- Q^T / K^T column tiles + RoPE ---
                for ct in range(CT):
                    if lch == 0 and ct == 0:
                        wt = wt_pre
                    else:
                        wt = ph1.tile([128, DC * 128], DT_P1, name="wt", tag="wt", bufs=3)
                        nc.sync.dma_start(wt[:], Wqkp[:, ct, :])
                    qk_ps = ps1.tile([128, 512], F32, name="qk_ps", tag="mm512", bufs=4)
                    for dc in range(DC):
                        nc.tensor.matmul(
                            qk_ps[:], wt[:, dc * 128:(dc + 1) * 128], xs[:, dc, :],
                            start=(dc == 0), stop=(dc == DC - 1),
                        )
                    # RoPE: qr = qk*cos + rot(qk)*sin_signed, written resident
                    if os.environ.get("NO_ROPE") == "1":
                        nc.scalar.copy(qkT_sb[:, ct, ls], qk_ps[:])
                    else:
                        rot = ph1.tile([128, 512], F32, name="rot", tag="rot", bufs=2)
                        nc.scalar.copy(rot[0:64, :], qk_ps[64:128, :])
                        nc.scalar.copy(rot[64:128, :], qk_ps[0:64, :])
                        t1 = ph1.tile([128, 512], F32, name="t1", tag="t1", bufs=2)
                        nc.vector.tensor_mul(t1[:], qk_ps[:], cos_sb[:])
                        t2 = ph1.tile([128, 512], F32, name="t2", tag="t2", bufs=2)
                        nc.vector.tensor_mul(t2[:], rot[:], sin_sb[:])
                        nc.vector.tensor_add(qkT_sb[:, ct, ls], t1[:], t2[:])
                    if lch == 0 and ct == 0:
                        # aux/persistent loads trickle on the gpsimd DMA ring,
                        # off the hot sync ring carrying wt/xs
                        nc.gpsimd.dma_start(wv_sb[:], Wvp[:, :])
                        nc.gpsimd.dma_start(wl_sb[:], Wlp[:, :])
                        nc.gpsimd.dma_start(bl_sb[:], blv[:, :])
                        nc.gpsimd.dma_start(mask_sb[:], maskT[:, :])
                        nc.gpsimd.dma_start(ones_sb[:], onesin[:, :])
                        nc.gpsimd.dma_start(wo_sb[:], Wop[:, :, :])

                # --- V tiles ---
                for lt in ([] if os.environ.get("NO_V") == "1" else range(4)):
                    v_ps = ps1.tile([128, 512], F32, name="v_ps", tag="mm512", bufs=4)
                    for dc in range(DC):
                        nc.tensor.matmul(
                            v_ps[:], xs[:, dc, lt * 128:(lt + 1) * 128],
                            wv_sb[:, dc * 512:(dc + 1) * 512],
                            start=(dc == 0), stop=(dc == DC - 1),
                        )
                    nc.scalar.copy(v_res[:, lch * 4 + lt, :], v_ps[:])

                # --- lambda ---
                if os.environ.get("NO_LAM") == "1":
                    continue
                lam_ps = ps1.tile([128, 512], F32, name="lam_ps", tag="mm512", bufs=4)
                for dc in range(DC):
                    nc.tensor.matmul(
                        lam_ps[:], wl_sb[:, dc * 128:(dc + 1) * 128], xs[:, dc, :],
                        start=(dc == 0), stop=(dc == DC - 1),
                    )
                nc.scalar.activation(
                    lam_sb[:, ls], lam_ps[0:NKV, :],
                    mybir.ActivationFunctionType.Sigmoid, bias=bl_sb[0:NKV, 0:1],
                )

        if dbg is not None and os.environ.get("NO_LAM") != "1":
            nc.sync.dma_start(dbg[0:NKV, :], lam_sb[:, :])
        trunc = os.environ.get("KTRUNC") == "1"
        # ================= Phase 2: causal attention per head pair =================
        # Two q-tiles (a "superblock": A=2sb, B=2sb+1) are processed at once so
        # every moving operand is 512 wide: columns ordered (qtile, head, l) =
        # [A.h0 | A.h1 | B.h0 | B.h1]. Per k-chunk: one S matmul [128,512], one
        # exp, one ctx matmul, one rowsum matmul. Chunk kc==A is full width but
        # masks its [0:256] half; chunk kc==B covers only [256:512]. Two kv
        # heads are processed concurrently (instruction-interleaved) so each
        # head's exp latency hides behind the other head's matmuls; ctx/rs for
        # group g are emitted after the S matmuls of group g+1. The two heads
        # share one rowsum PSUM bank (partitions 0 and 64), and normalization
        # tails are deferred into the next superblock.
        with tc.tile_pool(name="ph2", bufs=1) as ph2, \
                tc.tile_pool(name="ps2", bufs=1, space="PSUM") as ps2:
            pend_norm = []

            def emit_norm(st):
                ctx_ps, rs_sb, qtA_, p_, lam0_ = st
                recip = ph2.tile([1, 2, 256], F32, name="recip", tag="recip", bufs=2)
                nc.vector.reciprocal_approx_fast(
                    recip.rearrange("p t l -> p (t l)"), rs_sb[:]
                )
                cs = ph2.tile([1, 2, 256], F32R, name="cs", tag="cs", bufs=2)
                nc.vector.tensor_copy(cs[:, :, 0:128], recip[:, :, 0:128])
                nc.vector.tensor_mul(
                    cs[:, :, 128:256], recip[:, :, 128:256],
                    lam0_[:, qtA_ * 128:(qtA_ + 2) * 128].rearrange(
                        "p (t l) -> p t l", t=2
                    ),
                )
                b_ps = ps2.tile([128, 512], F32, name="b_ps", tag="bps", bufs=1)
                nc.tensor.matmul(
                    b_ps[:], ones_sb[0:1, :], cs.rearrange("p t l -> p (t l)"),
                    start=True, stop=True,
                )
                b_sb = ph2.tile([128, 2, 256], F32, name="b_sb", tag="bsb", bufs=2)
                nc.vector.tensor_copy(b_sb.rearrange("p t l -> p (t l)"), b_ps[:])
                ctx3 = ctx_ps.rearrange("p (t l) -> p t l", t=2)
                t0 = ph2.tile([128, 2, 128], F32, name="t0", tag="t0", bufs=2)
                nc.vector.tensor_mul(t0[:], ctx3[:, :, 0:128], b_sb[:, :, 0:128])
                t1b = ph2.tile([128, 2, 128], F32, name="t1b", tag="t1b", bufs=2)
                nc.vector.tensor_mul(t1b[:], ctx3[:, :, 128:256], b_sb[:, :, 128:256])
                nc.vector.tensor_sub(
                    diffT[:, p_, qtA_ * 128:(qtA_ + 2) * 128],
                    t0.rearrange("p t l -> p (t l)"),
                    t1b.rearrange("p t l -> p (t l)"),
                )

            nc.vector.tensor_copy(ones_att[:], ones_sb[:, 0:1])
            lam0s = []
            for p in range(NKV):
                lam0 = ph2.tile([1, L], F32, name="lam0", tag="lam0", bufs=NKV)
                nc.gpsimd.dma_start(lam0[:], lam_sb[p:p + 1, :])
                lam0s.append(lam0)

            def emit_block(st):
                ctx_ps, rs_ps, rsrow, e_sb, j, kc, qtB, off, wid, p_ = st
                nc.tensor.matmul(
                    ctx_ps[:, off:off + wid],
                    v_res[:, kc, p_ * 128:(p_ + 1) * 128],
                    e_sb[:, j, off:off + wid],
                    start=(kc == 0), stop=(kc == qtB), skip_group_check=True,
                )
                nc.tensor.matmul(
                    rs_ps[rsrow:rsrow + 1, off:off + wid], ones_att[:, 0:1],
                    e_sb[:, j, off:off + wid],
                    start=(kc == 0), stop=(kc == qtB), skip_group_check=True,
                )

            for sb in ([] if trunc else range(LT // 2)):
                qtA, qtB = 2 * sb, 2 * sb + 1
                groups = []
                kcs = list(range(qtB + 1))
                for gi in range(0, len(kcs), 2):
                    groups.append(kcs[gi:gi + 2])
                for pp in (0, 2):
                    pair = (pp, pp + 1)
                    rs_ps = ps2.tile([128, 512], F32, name="rs_ps", tag="small", bufs=1)
                    sts = {}
                    for idx, p in enumerate(pair):
                        sts[p] = {
                            "ctx": ps2.tile([128, 512], F32, name="ctx_ps", tag="ctx", bufs=2),
                            "rsrow": 64 * idx,
                            "rhs_full": qkT_sb[
                                :, 2 * p:2 * p + 2, qtA * 128:(qtA + 2) * 128
                            ].rearrange("pp h (t l) -> pp t h l", t=2),
                            "rhs_half": qkT_sb[
                                :, 2 * p:2 * p + 2, qtB * 128:(qtB + 1) * 128
                            ],
                            "pend": [],
                        }
                    npop = 0
                    for gk, grp in enumerate(groups):
                        for p in pair:
                            st = sts[p]
                            s_ps = ps2.tile([128, 2, 512], F32, name="s_ps", tag="s2", bufs=2)
                            segs = []
                            for j, kc in enumerate(grp):
                                off, wid = (256, 256) if kc == qtB else (0, 512)
                                rhs = st["rhs_half"] if kc == qtB else st["rhs_full"]
                                nc.tensor.matmul(
                                    s_ps[:, j, off:off + wid],
                                    qkT_sb[:, NQ + p, kc * 128:(kc + 1) * 128],
                                    rhs,
                                    start=True, stop=True, skip_group_check=True,
                                )
                                segs.append((j, kc, off, wid))
                            st["sps"] = s_ps
                            st["segs"] = segs
                        # deferred normalization tails from the previous unit
                        if pend_norm and npop < 2:
                            emit_norm(pend_norm.pop(0))
                            npop += 1
                        for p in pair:
                            st = sts[p]
                            # ctx/rowsum of the previous group (exp done by now)
                            while st["pend"]:
                                emit_block(st["pend"].pop(0))
                            s_ps, segs = st["sps"], st["segs"]
                            e_sb = ph2.tile([128, 2, 512], DT_ATT, name="e_sb", tag="e", bufs=4)
                            if len(segs) == 2 and segs[0][3] == 512 and segs[1][3] == 512:
                                nc.scalar.activation(
                                    e_sb.rearrange("p a b -> p (a b)"),
                                    s_ps.rearrange("p a b -> p (a b)"),
                                    mybir.ActivationFunctionType.Exp, scale=SCALE,
                                )
                            else:
                                for j, kc, off, wid in segs:
                                    nc.scalar.activation(
                                        e_sb[:, j, off:off + wid],
                                        s_ps[:, j, off:off + wid],
                                        mybir.ActivationFunctionType.Exp, scale=SCALE,
                                    )
                            for j, kc, off, wid in segs:
                                if kc == qtA:
                                    nc.vector.tensor_mul(
                                        e_sb[:, j, 0:256], e_sb[:, j, 0:256], mask_sb[:]
                                    )
                                elif kc == qtB:
                                    nc.vector.tensor_mul(
                                        e_sb[:, j, 256:512], e_sb[:, j, 256:512], mask_sb[:]
                                    )
                                st["pend"].append((
                                    st["ctx"], rs_ps, st["rsrow"], e_sb, j, kc,
                                    qtB, off, wid, p,
                                ))
                    for p in pair:
                        for st_ in sts[p]["pend"]:
                            emit_block(st_)
                    if pend_norm and npop < 2:
                        emit_norm(pend_norm.pop(0))
                        npop += 1
                    for idx, p in enumerate(pair):
                        rs_sb = ph2.tile([1, 512], F32, name="rs_sb", tag="rs_sb", bufs=4)
                        if idx == 0:
                            nc.vector.tensor_copy(rs_sb[:], rs_ps[0:1, :])
                        else:
                            nc.scalar.copy(rs_sb[:], rs_ps[64:65, :])
                        pend_norm.append((sts[p]["ctx"], rs_sb, qtA, p, lam0s[p]))
            while pend_norm:
                emit_norm(pend_norm.pop(0))

        # ================= Phase 3: output projection =================
        with tc.tile_pool(name="ph3", bufs=1) as ph3, \
                tc.tile_pool(name="ps3", bufs=1, space="PSUM") as ps3:
            for ot in ([] if trunc else range(LT)):
                for qch in range(LCH):
                    o_ps = ps3.tile([128, 512], F32, name="o_ps", tag="mm512", bufs=4)
                    for p in range(NKV):
                        nc.tensor.matmul(
                            o_ps[:],
                            wo_sb[:, p, ot * 128:(ot + 1) * 128],
                            diffT[:, p, qch * 512:(qch + 1) * 512],
                            start=(p == 0), stop=(p == NKV - 1),
                        )
                    o_sb = ph3.tile([128, 512], DT_OUT, name="o_sb", tag="osb", bufs=4)
                    nc.scalar.copy(o_sb[:], o_ps[:])
                    nc.sync.dma_start(
                        outT[ot * 128:(ot + 1) * 128, qch * 512:(qch + 1) * 512], o_sb[:]
                    )

    nc.finalize()
    return nc


def _host_tables():
    half = DH // 2
    inv_freq = 1.0 / (ROPE_BASE ** (np.arange(0, half, dtype=np.float64) * 2.0 / DH))
    freqs = np.arange(L, dtype=np.float64)[:, None] * inv_freq[None, :]  # [L, half]
    emb = np.concatenate([freqs, freqs], axis=-1)  # [L, DH]
    cosT = np.ascontiguousarray(np.cos(emb).T.astype(np.float32))  # [DH, L]
    sinT = np.sin(emb).T.astype(np.float32)
    sinTs = np.concatenate([-sinT[:half], sinT[half:]], axis=0)
    sinTs = np.ascontiguousarray(sinTs.astype(np.float32))
    tri = np.triu(np.ones((128, 128), dtype=np.float32))  # keep k' <= q'
    maskT = np.ascontiguousarray(np.concatenate([tri, tri], axis=1))
    ones = np.ones((128, 128), dtype=np.float32)
    return cosT, sinTs, maskT, ones


def _pack_rows(w, inner):
    """[D, C] -> [128, (DC, C)] with row dc*128+p on partition p, contiguous
    per-partition lines (C = inner)."""
    d, c = w.shape
    assert c == inner and d == D
    return np.ascontiguousarray(
        w.reshape(DC, 128, inner).transpose(1, 0, 2).reshape(128, DC * inner)
    )


_NC_CACHE = []


def kernel(x, Wq, Wk, Wv, Wl, bl, Wo):
    x = np.asarray(x, dtype=np.float32)
    Wq = np.asarray(Wq, dtype=np.float32)
    Wk = np.asarray(Wk, dtype=np.float32)
    Wv = np.asarray(Wv, dtype=np.float32)
    Wl = np.asarray(Wl, dtype=np.float32)
    bl = np.asarray(bl, dtype=np.float32)
    Wo = np.asarray(Wo, dtype=np.float32)

    cosT, sinTs, maskT, ones = _host_tables()
    Wq3 = Wq.reshape(D, 2 * NH, DH)
    Wk3 = Wk.reshape(D, NH, DH)

    in_maps = []
    for c in range(8):
        b, g = divmod(c, G)
        wq_s = Wq3[:, 8 * g:8 * g + NQ, :].reshape(D, NQ * DH)
        wk_s = Wk3[:, G * g:G * g + NKV, :].reshape(D, NKV * DH)
        wqk = np.concatenate([wq_s, wk_s], axis=1).astype(NP_P1)  # [D, CQK]
        # pack: Wqkp[p, ct, dc*128+cc] = wqk[dc*128+p, ct*128+cc]
        wqkp = np.ascontiguousarray(
            wqk.reshape(DC, 128, CT, 128).transpose(1, 2, 0, 3).reshape(128, CT, DC * 128)
        )
        wv_s = Wv[:, DH * G * g:DH * G * g + NKV * DH].astype(NP_P1)
        wl_s = np.pad(Wl[:, G * g:G * g + NKV], ((0, 0), (0, 128 - NKV))).astype(NP_P1)
        wo_s = Wo[512 * g:512 * (g + 1), :].astype(NP_P3)  # [NKV*128, D]
        wop = np.ascontiguousarray(wo_s.reshape(NKV, 128, D).transpose(1, 0, 2))
        in_maps.append({
            "xT": np.ascontiguousarray(x[b].T).astype(NP_P1),
            "Wqkp": wqkp,
            "Wvp": _pack_rows(wv_s, NKV * DH),
            "Wlp": _pack_rows(wl_s, 128),
            "blv": np.ascontiguousarray(np.pad(bl[G * g:G * g + NKV], (0, 128 - NKV)).reshape(128, 1)),
            "Wop": wop,
            "cosT": cosT,
            "sinTs": sinTs,
            "maskT": maskT.astype(NP_ATT),
            "onesin": ones,
        })

    if not _NC_CACHE:
        _NC_CACHE.append(build_kernel())
    nc = _NC_CACHE[0]
    res = run_bass_kernel_spmd(nc, in_maps, core_ids=list(range(8)))

    out = np.empty((B, L, D), dtype=np.float32)
    for b in range(B):
        acc = res.results[4 * b]["outT"].astype(np.float32)
        for g in range(1, G):
            acc += res.results[4 * b + g]["outT"].astype(np.float32)
        out[b] = acc.T
    return out
